# revision 3
# baseline (speedup 1.0000x reference)
"""ClassAttention Trainium2 kernel (Bass/Tile), data-parallel over batch on 8 cores.

Math (per batch b):
  q = x[b,0] @ W_q                      -> [H, D]
  k = x[b] @ W_k ; v = x[b] @ W_v       (W_k/W_v = halves of W_kv)
  scores = (q * SCALE) . k  per head    -> [H, N]
  attn = softmax(scores, axis=N)
  cls = attn @ v (per head)             -> [H*D]
  out[b] = cls @ W_proj + b_proj

Two algebraic tricks eliminate both giant matmuls (x@W_k and x@W_v):
 1. Fold q into the weights so k is never materialized:
      Q'_b[64h+d, h] = q_b[h,d] * SCALE   (block-diagonal scatter, [C, H])
      G_b = W_k @ Q'_b                    ([C, H], per batch)
      scores^T = G_b^T @ x_b^T            (16-column matmul)
 2. Reassociate the value path: cls = (attn @ x) @ W_v
      y_b = attn_b @ x_b                  ([H, C], contraction over tokens —
                                           uses x tiles in natural layout)
      cls  = diag-blocks of (W_v^T y^T)   (one 128-col matmul for all batches)

Remaining x^T tiles (scores path only) come from PE transposes.
All matmuls in bf16 (cast during DMA), fp32 accumulation.
Each core handles 8 batches; no collectives. Host shards/concats.
"""

import numpy as np
from contextlib import ExitStack

B, N, C = 64, 1024, 1024
H, D = 16, 64
SCALE = D**-0.5
NCORES = 8
BL = B // NCORES  # batches per core
CCH = C // 128  # chunks over any 1024-dim
MT = N // 128  # token chunks per batch

_BUILT = {}


def _build_module():
    import concourse.mybir as mybir
    import concourse.tile as tile
    from concourse import bacc
    from concourse.masks import make_identity

    f32 = mybir.dt.float32
    bf16 = mybir.dt.bfloat16
    AF = mybir.ActivationFunctionType

    nc = bacc.Bacc("TRN2", target_bir_lowering=False, debug=False)

    x_d = nc.dram_tensor("x", [BL, N, C], f32, kind="ExternalInput")
    wkv_d = nc.dram_tensor("W_kv", [C, 2 * H * D], f32, kind="ExternalInput")
    wq_d = nc.dram_tensor("W_q", [C, H * D], f32, kind="ExternalInput")
    wp_d = nc.dram_tensor("W_proj", [H * D, C], f32, kind="ExternalInput")
    bp_d = nc.dram_tensor("b_proj", [C], f32, kind="ExternalInput")
    out_d = nc.dram_tensor("out", [BL, C], f32, kind="ExternalOutput")

    with tile.TileContext(nc) as tc, ExitStack() as ctx:
        const = ctx.enter_context(tc.tile_pool(name="const", bufs=1))
        work = ctx.enter_context(tc.tile_pool(name="work", bufs=2))
        xpool = ctx.enter_context(tc.tile_pool(name="xp", bufs=3))
        xtpool = ctx.enter_context(tc.tile_pool(name="xtp", bufs=3))
        apool = ctx.enter_context(tc.tile_pool(name="ap", bufs=9))
        ps_t = ctx.enter_context(tc.tile_pool(name="ps_t", bufs=3, space="PSUM"))
        ps_acc = ctx.enter_context(tc.tile_pool(name="ps_acc", bufs=4, space="PSUM"))

        # ---------------- persistent tiles ----------------
        ident_bf = const.tile([128, 128], bf16, tag="ident_bf")
        make_identity(nc, ident_bf[:, :])
        ident_f32 = const.tile([128, 128], f32, tag="ident_f32")
        make_identity(nc, ident_f32[:, :])

        # weights, cast fp32->bf16 during DMA (SWDGE)
        wv_sb = const.tile([128, CCH, 1024], bf16, tag="wv")  # [p(c), cc, c']
        nc.gpsimd.dma_start(
            out=wv_sb[:, :, :],
            in_=wkv_d[:, 1024:2048].rearrange("(cc p) j -> p cc j", p=128),
        )
        wq_sb = const.tile([128, CCH, 1024], bf16, tag="wq")  # [p, cc, m]
        nc.gpsimd.dma_start(
            out=wq_sb[:, :, :], in_=wq_d[:, :].rearrange("(cc p) m -> p cc m", p=128)
        )
        wk_stage = const.tile([128, CCH, 1024], bf16, tag="wkstage")  # [p, cc, j]
        nc.gpsimd.dma_start(
            out=wk_stage[:, :, :],
            in_=wkv_d[:, 0:1024].rearrange("(cc p) j -> p cc j", p=128),
        )
        wp_sb = const.tile([128, CCH, 1024], bf16, tag="wp")  # [p, cc, o]
        nc.gpsimd.dma_start(
            out=wp_sb[:, :, :], in_=wp_d[:, :].rearrange("(cc p) o -> p cc o", p=128)
        )
        b_sb = const.tile([128, CCH], f32, tag="b")  # [p, mo]
        nc.gpsimd.dma_start(
            out=b_sb[:, :], in_=bp_d[:].rearrange("(mo p) -> p mo", p=128)
        )
        # CLS-token columns of x, transposed: [p(c), cc, b]
        xclsT = const.tile([128, CCH, BL], bf16, tag="xclsT")
        for cc in range(CCH):
            nc.gpsimd.dma_start(
                out=xclsT[:, cc, :],
                in_=x_d[:, 0, cc * 128 : (cc + 1) * 128].rearrange("b p -> p b"),
            )

        # ---------------- W_k^T via PE transpose ----------------
        wkT = const.tile([128, CCH, 1024], bf16, tag="wkT")  # [p(j), jc, c]
        for jc in range(CCH):
            for cc in range(CCH):
                pst = ps_t.tile([128, 128], bf16, tag="ps_tr")
                nc.tensor.transpose(
                    pst[:, :],
                    wk_stage[:, cc, jc * 128 : (jc + 1) * 128],
                    ident_bf[:, :],
                )
                if cc % 2 == 0:
                    nc.vector.tensor_copy(wkT[:, jc, cc * 128 : (cc + 1) * 128], pst[:, :])
                else:
                    nc.scalar.copy(wkT[:, jc, cc * 128 : (cc + 1) * 128], pst[:, :])

        # ---------------- q for all batches; scatter into Q' ----------------
        # Q'[p, jc, b*H+h]; block-diagonal with SCALE folded in
        qp_sb = const.tile([128, CCH, BL * H], bf16, tag="qp")
        nc.vector.memset(qp_sb[:, :, :], 0.0)
        for m in range(CCH):
            psq = ps_acc.tile([128, BL], f32, tag="ps_acc")
            for cc in range(CCH):
                nc.tensor.matmul(
                    psq[:, :],
                    wq_sb[:, cc, m * 128 : (m + 1) * 128],
                    xclsT[:, cc, :],
                    start=(cc == 0),
                    stop=(cc == CCH - 1),
                )
            # head of c' = 128*m + p is 2m + p//64
            qv = qp_sb[:, m, :].rearrange("p (b h) -> p h b", h=H)
            nc.scalar.activation(qv[0:64, 2 * m, :], psq[0:64, :], AF.Copy, scale=SCALE)
            nc.scalar.activation(
                qv[64:128, 2 * m + 1, :], psq[64:128, :], AF.Copy, scale=SCALE
            )

        # ---------------- G = W_k @ Q' (all batches) ----------------
        g_sb = const.tile([128, CCH, BL * H], bf16, tag="g")  # [p(c), cc, b*H+h]
        for cc in range(CCH):
            psg = ps_acc.tile([128, BL * H], f32, tag="ps_acc")
            for jc in range(CCH):
                nc.tensor.matmul(
                    psg[:, :],
                    wkT[:, jc, cc * 128 : (cc + 1) * 128],
                    qp_sb[:, jc, :],
                    start=(jc == 0),
                    stop=(jc == CCH - 1),
                )
            nc.vector.tensor_copy(g_sb[:, cc, :], psg[:, :])

        # y^T for all batches: [p(c), cc, b*H+h]
        yT_all = const.tile([128, CCH, BL * H], bf16, tag="yT")
        out_all = const.tile([BL, C], f32, tag="out_all")

        # ---------------- main loop over batches ----------------
        for b in range(BL):
            x_sb = xpool.tile([128, MT, C], bf16, tag="x")
            nc.gpsimd.dma_start(
                out=x_sb[:, :, :], in_=x_d[b, :, :].rearrange("(t p) c -> p t c", p=128)
            )
            sT = work.tile([H, N], f32, tag="scoresT")
            for t in range(MT):
                xt = xtpool.tile([128, CCH, 128], bf16, tag="xt")
                for cc in range(CCH):
                    pst = ps_t.tile([128, 128], bf16, tag="ps_tr")
                    nc.tensor.transpose(
                        pst[:, :], x_sb[:, t, cc * 128 : (cc + 1) * 128], ident_bf[:, :]
                    )
                    if cc % 2 == 0:
                        nc.vector.tensor_copy(xt[:, cc, :], pst[:, :])
                    else:
                        nc.scalar.copy(xt[:, cc, :], pst[:, :])
                ps_s = ps_acc.tile([H, 128], f32, tag="ps_acc")
                for cc in range(CCH):
                    nc.tensor.matmul(
                        ps_s[:, :],
                        g_sb[:, cc, b * H : (b + 1) * H],
                        xt[:, cc, :],
                        start=(cc == 0),
                        stop=(cc == CCH - 1),
                    )
                nc.vector.tensor_copy(sT[:, t * 128 : (t + 1) * 128], ps_s[:, :])

            # softmax over N (free dim of sT)
            negm = work.tile([H, 1], f32, tag="negm")
            nc.vector.reduce_max(
                negm[:, :], sT[:, :], axis=mybir.AxisListType.X, negate=True
            )
            expT = work.tile([H, N], f32, tag="expT")
            sume = work.tile([H, 1], f32, tag="sume")
            nc.scalar.activation(
                expT[:, :], sT[:, :], AF.Exp, bias=negm[:, :], accum_out=sume[:, :]
            )
            rs = work.tile([H, 1], f32, tag="rs")
            nc.vector.reciprocal(rs[:, :], sume[:, :])
            attnT = work.tile([H, N], bf16, tag="attnT")
            nc.vector.tensor_scalar_mul(attnT[:, :], expT[:, :], rs[:, :])

            attn_tiles = []
            for t in range(MT):
                ps_a = ps_t.tile([128, H], bf16, tag="ps_tr")
                nc.tensor.transpose(
                    ps_a[:, :], attnT[:, t * 128 : (t + 1) * 128], ident_bf[0:H, 0:H]
                )
                a_sb = apool.tile([128, H], bf16, tag="attn")
                nc.vector.tensor_copy(a_sb[:, :], ps_a[:, :])
                attn_tiles.append(a_sb)

            # y_b^T = (attn_b @ x_b)^T, per c-chunk; contraction over tokens
            for cc in range(CCH):
                ps_y = ps_acc.tile([128, H], f32, tag="ps_acc")
                for t in range(MT):
                    nc.tensor.matmul(
                        ps_y[:, :],
                        x_sb[:, t, cc * 128 : (cc + 1) * 128],
                        attn_tiles[t][:, :],
                        start=(t == 0),
                        stop=(t == MT - 1),
                    )
                nc.scalar.copy(yT_all[:, cc, b * H : (b + 1) * H], ps_y[:, :])

        # ---------------- cls for all batches: diag blocks of W_v^T @ y^T ----
        # cls vectors (bf16): [p(c'), m, b]
        clsT = const.tile([128, CCH, BL], bf16, tag="clsT")
        for m in range(CCH):
            ps_c = ps_acc.tile([128, BL * H], f32, tag="ps_acc")
            for cc in range(CCH):
                nc.tensor.matmul(
                    ps_c[:, :],
                    wv_sb[:, cc, m * 128 : (m + 1) * 128],
                    yT_all[:, cc, :],
                    start=(cc == 0),
                    stop=(cc == CCH - 1),
                )
            # head of c' = 128m + p is 2m + p//64: pick column b*H + head
            pv = ps_c[:, :].rearrange("p (b h) -> p h b", h=H)
            nc.scalar.copy(clsT[0:64, m, :], pv[0:64, 2 * m, :])
            nc.scalar.copy(clsT[64:128, m, :], pv[64:128, 2 * m + 1, :])

        # ---------------- projection + bias for all batches ----------------
        for mo in range(CCH):
            ps_o = ps_acc.tile([128, BL], f32, tag="ps_acc")
            for cc in range(CCH):
                nc.tensor.matmul(
                    ps_o[:, :],
                    wp_sb[:, cc, mo * 128 : (mo + 1) * 128],
                    clsT[:, cc, :],
                    start=(cc == 0),
                    stop=(cc == CCH - 1),
                )
            tmp = work.tile([128, BL], f32, tag="tmpo")
            nc.vector.tensor_scalar_add(tmp[:, :], ps_o[:, :], b_sb[:, mo : mo + 1])
            ps_ot = ps_t.tile([BL, 128], f32, tag="ps_tr")
            nc.tensor.transpose(ps_ot[:, :], tmp[:, :], ident_f32[:, :])
            nc.vector.tensor_copy(out_all[:, mo * 128 : (mo + 1) * 128], ps_ot[:, :])

        nc.sync.dma_start(out=out_d[:, :], in_=out_all[:, :])

    nc.compile()
    return nc


def get_module():
    if "nc" not in _BUILT:
        _BUILT["nc"] = _build_module()
    return _BUILT["nc"]


def kernel(x, W_kv, W_q, W_proj, b_proj):
    from concourse.bass_utils import run_bass_kernel_spmd

    x = np.ascontiguousarray(np.asarray(x, dtype=np.float32))
    W_kv = np.ascontiguousarray(np.asarray(W_kv, dtype=np.float32))
    W_q = np.ascontiguousarray(np.asarray(W_q, dtype=np.float32))
    W_proj = np.ascontiguousarray(np.asarray(W_proj, dtype=np.float32))
    b_proj = np.ascontiguousarray(np.asarray(b_proj, dtype=np.float32))

    nc = get_module()
    in_maps = []
    for core in range(NCORES):
        in_maps.append(
            {
                "x": x[core * BL : (core + 1) * BL],
                "W_kv": W_kv,
                "W_q": W_q,
                "W_proj": W_proj,
                "b_proj": b_proj,
            }
        )
    res = run_bass_kernel_spmd(nc, in_maps, core_ids=list(range(NCORES)))
    outs = [res.results[core]["out"] for core in range(NCORES)]
    return np.concatenate(outs, axis=0).reshape(B, 1, C).astype(np.float32)
